# revision 21
# baseline (speedup 1.0000x reference)
"""Trainium2 Bass kernel for nn_DiffusionLoss (retrieval_knn), 8-core SPMD.

Host shards the full inputs across 8 NeuronCores (core = 2*batch + half; each
core owns 4096 of its batch's 8192 points, and its own rows are permuted to
the FRONT of the column order so every per-core slice is compile-time static).

Per-core device algorithm:
  continuity:
    fwd  G'[i,j] = 2 x_i.x_j - |x_j|^2          (K=4 float32r matmul)
         -> ACT drains PSUM to SBUF, diag -> -1e30, DVE max8 -> tau (8th NN)
    bwd  H[j,i] = 2 x_j.x_i - |x_j|^2 - (tau_i-EPS)   (K=6 f32r, j on parts;
         tau enters as an exact hi+lo float32r pair against two ones rows)
         -> ACT Sigmoid(1e6*H) drains PSUM to a 0/1 fp16 transposed mask
    mask-mm  maskT @ [x,y,z,|x|^2,1] fp16 -> s1(3), s2, cnt per row
         (self included by construction; subtracted exactly afterward using
          the same fp16 table values)
    finalize sum_i 8*(s2/cnt - |s1|^2/cnt^2) per partition
  recon/percep: sub + ACT Square accumulate
  boundary: row-max of 2 c1.c2 - |c2_j|^2 in plain f32, min_d =
    sqrt(relu(|c1|^2 - max)), masked sum + count.
Host: combines per-core partials into (recon, percep, cont, bnd, total).
"""
import numpy as np

B, N, HALF = 4, 8192, 4096
NSTRIP = 32            # 128-row strips per core
NJT = 64               # j-tiles of 128
EPS = 3e-5
SIG_SCALE = 1.0e6
NEG_BIG = -1.0e30

_COMPILED = None
REPS_TIMING = 4


def _build_core(reps=1):
    import concourse.bass as bass
    import concourse.mybir as mybir
    import concourse.tile as tile

    f32 = mybir.dt.float32
    f32r = mybir.dt.float32r
    f16 = mybir.dt.float16
    AF = mybir.ActivationFunctionType
    ALU = mybir.AluOpType
    AX = mybir.AxisListType

    nc = bass.Bass()
    ptb = nc.dram_tensor("ptb", [48, 512], f32, kind="ExternalInput")
    prow = nc.dram_tensor("prow", [128, 96], f32, kind="ExternalInput")
    trow = nc.dram_tensor("trow", [128, 96], f32, kind="ExternalInput")
    pfeat = nc.dram_tensor("pfeat", [128, 8], f32, kind="ExternalInput")
    tfeat = nc.dram_tensor("tfeat", [128, 8], f32, kind="ExternalInput")
    c1t = nc.dram_tensor("c1t", [3, 512], f32, kind="ExternalInput")
    c1r = nc.dram_tensor("c1r", [128, 12], f32, kind="ExternalInput")
    c2b = nc.dram_tensor("c2b", [24, 512], f32, kind="ExternalInput")
    out = nc.dram_tensor("out", [128, 8], f32, kind="ExternalOutput")

    with tile.TileContext(nc) as tc:
        with tc.tile_pool(name="persist", bufs=1) as pp, \
             tc.tile_pool(name="strip", bufs=2) as sp, \
             tc.tile_pool(name="mask", bufs=4) as mp, \
             tc.tile_pool(name="small", bufs=1) as smp, \
             tc.tile_pool(name="psF", bufs=2, space="PSUM") as psF, \
             tc.tile_pool(name="psB", bufs=2, space="PSUM") as psB, \
             tc.tile_pool(name="psS", bufs=1, space="PSUM") as psS:

            fill_big = nc.gpsimd.to_reg(NEG_BIG)
            fill_zero = nc.gpsimd.to_reg(0.0)
            for _rep in range(reps):
                # ---------------- setup ----------------
                t_ptb = pp.tile([48, 512], f32)
                nc.sync.dma_start(t_ptb[:], ptb[:])

                sq48 = smp.tile([48, 512], f32, tag="dff")
                nc.scalar.activation(sq48[:], t_ptb[:], AF.Square)
                sqy0 = smp.tile([16, 512], f32, tag="pr")
                nc.sync.dma_start(sqy0[:], sq48[16:32, :])
                sqz0 = smp.tile([16, 512], f32, tag="tr")
                nc.sync.dma_start(sqz0[:], sq48[32:48, :])
                x2p = pp.tile([16, 512], f32)
                nc.vector.tensor_add(x2p[:], sq48[0:16, :], sqy0[:])
                nc.vector.tensor_add(x2p[:], x2p[:], sqz0[:])
                x2n = pp.tile([16, 512], f32r)
                nc.vector.tensor_scalar_mul(x2n[:], x2p[:], -1.0)
                db2 = pp.tile([48, 512], f32r)
                nc.vector.tensor_scalar_mul(db2[:], t_ptb[:], 2.0)
                a48 = pp.tile([48, 512], f32r)
                nc.vector.tensor_copy(a48[:], t_ptb[:])

                ones1 = pp.tile([1, 512], f32)
                nc.vector.memset(ones1[:], 1.0)

                # T5a [4,8192] f32r rows x,y,z,1 (fwd stationary / bwd src)
                T5a = pp.tile([4, N], f32r)
                for c in range(3):
                    nc.sync.dma_start(T5a[c:c + 1, :],
                                      a48[c * 16:(c + 1) * 16, :])
                for _c in range(16):
                    nc.sync.dma_start(
                        T5a[3:4, _c * 512:(_c + 1) * 512].bitcast(f32),
                        ones1[:])
                # T5b [6,8192] f32r rows 2x,2y,2z,-x2,1,1
                # (fwd moving rows 0:4 / bwd stationary rows 0:6)
                T5b = pp.tile([6, N], f32r)
                for c in range(3):
                    nc.sync.dma_start(T5b[c:c + 1, :],
                                      db2[c * 16:(c + 1) * 16, :])
                nc.sync.dma_start(T5b[3:4, :], x2n[:])
                for _r in (4, 5):
                    for _c in range(16):
                        nc.sync.dma_start(
                            T5b[_r:_r + 1, _c * 512:(_c + 1) * 512]
                            .bitcast(f32), ones1[:])

                # identities
                eye4 = pp.tile([4, 4], f32)
                nc.gpsimd.memset(eye4[:], 1.0)
                nc.gpsimd.affine_select(eye4[:], eye4[:], [[-1, 4]],
                                        ALU.is_equal, fill_zero,
                                        channel_multiplier=1)
                eye5 = pp.tile([5, 5], f32)
                nc.gpsimd.memset(eye5[:], 1.0)
                nc.gpsimd.affine_select(eye5[:], eye5[:], [[-1, 5]],
                                        ALU.is_equal, fill_zero,
                                        channel_multiplier=1)
                eye128 = pp.tile([128, 128], f32)
                nc.gpsimd.memset(eye128[:], 1.0)
                nc.gpsimd.affine_select(eye128[:], eye128[:], [[-1, 128]],
                                        ALU.is_equal, fill_zero,
                                        channel_multiplier=1)

                # T4w [4,8192] f32 rows x,y,z,x2 -> Wtab fp16 j-table
                T4w = sp.tile([4, N], f32, tag="strip")
                for c in range(3):
                    nc.sync.dma_start(T4w[c:c + 1, :],
                                      t_ptb[c * 16:(c + 1) * 16, :])
                nc.sync.dma_start(T4w[3:4, :], x2p[:])
                Wtab = pp.tile([128, NJT * 5], f16)
                for jt in range(NJT):
                    pw = psS.tile([128, 128], f32, tag="tp")
                    nc.tensor.transpose(pw[:, 0:4],
                                        T4w[:, jt * 128:(jt + 1) * 128],
                                        eye4[:])
                    nc.scalar.copy(Wtab[:, jt * 5:jt * 5 + 4], pw[:, 0:4])
                Wtab5 = Wtab[:].rearrange("p (j c) -> p j c", c=5)
                nc.vector.memset(Wtab5[:, :, 4:5], 1.0)

                # bwd moving T5bp [6,4096] f32r rows x,y,z,1,-tauhi,-taulo
                T5bp = pp.tile([6, HALF], f32r, tag="bigslot")
                nc.vector.tensor_copy(T5bp[0:3, :], T5a[0:3, 0:HALF])
                for _c in range(8):
                    nc.sync.dma_start(
                        T5bp[3:4, _c * 512:(_c + 1) * 512].bitcast(f32),
                        ones1[:])

                # SaTT [128, 32, 5] f32: per-row (s1, s2, cntF);
                # row i lives at (partition i%128, w=i//128)
                SaTT = pp.tile([128, 32, 5], f32)

                # ---------------- main loop ----------------
                for g in range(NSTRIP // 2):
                    for s in (2 * g, 2 * g + 1):
                        strip = sp.tile([128, N], f32, tag="strip")
                        for cs in range(16):
                            pF = psF.tile([128, 512], f32)
                            nc.tensor.matmul(
                                pF[:],
                                T5a[:, s * 128:(s + 1) * 128],
                                T5b[0:4, cs * 512:(cs + 1) * 512],
                                start=True, stop=True)
                            if cs % 2 == 1:
                                nc.vector.tensor_copy(
                                    strip[:, cs * 512:(cs + 1) * 512], pF[:])
                            else:
                                nc.scalar.copy(
                                    strip[:, cs * 512:(cs + 1) * 512], pF[:])
                        # self-distance -> -BIG on the diagonal block
                        nc.gpsimd.affine_select(
                            strip[:, s * 128:(s + 1) * 128],
                            strip[:, s * 128:(s + 1) * 128],
                            [[-1, 128]], ALU.not_equal, fill_big,
                            channel_multiplier=1)
                        top8 = smp.tile([128, 8], f32, tag="top8")
                        nc.vector.max(out=top8[:], in_=strip[:])
                        # tau -> row layout, split into f32r hi + lo
                        pT = psS.tile([128, 128], f32, tag="tp")
                        nc.tensor.transpose(pT[0:1, :], top8[:, 7:8],
                                            eye128[:])
                        taucol = smp.tile([1, 128], f32r, tag="taucol")
                        nc.scalar.activation(taucol[:], pT[0:1, :], AF.Copy,
                                             bias=EPS, scale=-1.0)
                        tauf = smp.tile([1, 128], f32, tag="tauf")
                        nc.scalar.activation(tauf[:], pT[0:1, :], AF.Copy,
                                             bias=EPS, scale=-1.0)
                        taulo = smp.tile([1, 128], f32r, tag="taulo")
                        nc.vector.tensor_tensor(taulo[:], tauf[:],
                                                taucol[:].bitcast(f32),
                                                op=ALU.subtract)
                        nc.sync.dma_start(
                            T5bp[4:5, s * 128:(s + 1) * 128], taucol[:])
                        nc.sync.dma_start(
                            T5bp[5:6, s * 128:(s + 1) * 128], taulo[:])

                    # bwd for this group's 256 i-cols + mask-mm accumulation
                    pS = psS.tile([5, 256], f32, tag="pS")
                    for jb in range(16):
                        pB = psB.tile([128, 1024], f32)
                        for q in range(4):
                            jt = jb * 4 + q
                            nc.tensor.matmul(
                                pB[:, q * 256:(q + 1) * 256],
                                T5b[0:6, jt * 128:(jt + 1) * 128],
                                T5bp[0:6, g * 256:(g + 1) * 256],
                                start=True, stop=True)
                        mt = mp.tile([128, 1024], f16, tag="mt")
                        nc.scalar.activation(mt[:], pB[:], AF.Sigmoid,
                                             scale=SIG_SCALE)
                        for q in range(4):
                            jt = jb * 4 + q
                            nc.tensor.matmul(
                                pS[:, 0:256],
                                Wtab[:, jt * 5:(jt + 1) * 5],
                                mt[:, q * 256:(q + 1) * 256],
                                start=(jt == 0), stop=(jt == NJT - 1),
                                skip_group_check=(jt != 0))
                    pSs = smp.tile([5, 256], f32, tag="pSs")
                    nc.scalar.copy(pSs[:], pS[:])
                    for h2 in range(2):
                        w = g * 2 + h2
                        ptW = psS.tile([128, 128], f32, tag="tp")
                        nc.tensor.transpose(ptW[:, 0:5],
                                            pSs[:, h2 * 128:(h2 + 1) * 128],
                                            eye5[:])
                        nc.scalar.copy(SaTT[:, w, :], ptW[:, 0:5])

                # ---------------- continuity finalize ----------------
                # subtract self (exact: same fp16 W values the mask-mm saw)
                Wself = pp.tile([128, 32, 4], f32)
                nc.vector.tensor_copy(Wself[:], Wtab5[:, 0:32, 0:4])
                nc.vector.tensor_tensor(SaTT[:, :, 0:4], SaTT[:, :, 0:4],
                                        Wself[:], op=ALU.subtract)
                nc.vector.tensor_scalar_add(SaTT[:, :, 4], SaTT[:, :, 4],
                                            -1.0)
                fzT = pp.tile([128, 32, 4], f32)
                nc.vector.tensor_tensor(fzT[:, :, 0:3], SaTT[:, :, 0:3],
                                        SaTT[:, :, 0:3], op=ALU.mult)
                nc.vector.tensor_add(fzT[:, :, 0], fzT[:, :, 0],
                                     fzT[:, :, 1])
                nc.vector.tensor_add(fzT[:, :, 0], fzT[:, :, 0],
                                     fzT[:, :, 2])
                nc.vector.reciprocal(fzT[:, :, 1], SaTT[:, :, 4])
                nc.vector.tensor_tensor(fzT[:, :, 2], fzT[:, :, 0],
                                        fzT[:, :, 1], op=ALU.mult)
                nc.vector.tensor_tensor(fzT[:, :, 2], fzT[:, :, 2],
                                        fzT[:, :, 1], op=ALU.mult)
                nc.vector.tensor_tensor(fzT[:, :, 3], SaTT[:, :, 3],
                                        fzT[:, :, 1], op=ALU.mult)
                nc.vector.tensor_sub(fzT[:, :, 3], fzT[:, :, 3],
                                     fzT[:, :, 2])
                nc.vector.tensor_scalar_mul(fzT[:, :, 3], fzT[:, :, 3], 8.0)
                cont_p = pp.tile([128, 1], f32)
                nc.vector.tensor_reduce(cont_p[:], fzT[:, :, 3], axis=AX.X,
                                        op=ALU.add)

                # ---------------- recon / percep ----------------
                t_prow = smp.tile([128, 96], f32, tag="pr")
                nc.sync.dma_start(t_prow[:], prow[:])
                t_trow = smp.tile([128, 96], f32, tag="tr")
                nc.sync.dma_start(t_trow[:], trow[:])
                dif = smp.tile([128, 96], f32, tag="dif")
                nc.vector.tensor_sub(dif[:], t_prow[:], t_trow[:])
                rsc = smp.tile([128, 96], f32, tag="rsc")
                rec_acc = pp.tile([128, 1], f32)
                nc.scalar.activation(rsc[:], dif[:], AF.Square,
                                     accum_out=rec_acc[:])
                t_pf = smp.tile([128, 8], f32, tag="pf")
                nc.sync.dma_start(t_pf[:], pfeat[:])
                t_tf = smp.tile([128, 8], f32, tag="tf")
                nc.sync.dma_start(t_tf[:], tfeat[:])
                dff = smp.tile([128, 8], f32, tag="dff2")
                nc.vector.tensor_sub(dff[:], t_pf[:], t_tf[:])
                fsc = smp.tile([128, 8], f32, tag="fsc")
                per_acc = pp.tile([128, 1], f32)
                nc.scalar.activation(fsc[:], dff[:], AF.Square,
                                     accum_out=per_acc[:])

                # ---------------- boundary (plain f32) ----------------
                t_c2b = pp.tile([24, 512], f32)
                nc.sync.dma_start(t_c2b[:], c2b[:])
                sqc = pp.tile([24, 512], f32)
                nc.scalar.activation(sqc[:], t_c2b[:], AF.Square)
                sqcy = smp.tile([8, 512], f32, tag="dif")
                nc.sync.dma_start(sqcy[:], sqc[8:16, :])
                sqcz = smp.tile([8, 512], f32, tag="rsc")
                nc.sync.dma_start(sqcz[:], sqc[16:24, :])
                y2p = pp.tile([8, 512], f32)
                nc.vector.tensor_add(y2p[:], sqc[0:8, :], sqcy[:])
                nc.vector.tensor_add(y2p[:], y2p[:], sqcz[:])
                y2n = pp.tile([8, 512], f32)
                nc.vector.tensor_scalar_mul(y2n[:], y2p[:], -1.0)
                dc2 = pp.tile([24, 512], f32)
                nc.vector.tensor_scalar_mul(dc2[:], t_c2b[:], 2.0)
                T4c = pp.tile([6, HALF], f32, tag="bigslot")
                for c in range(3):
                    nc.sync.dma_start(T4c[c:c + 1, :],
                                      dc2[c * 8:(c + 1) * 8, :])
                nc.sync.dma_start(T4c[3:4, :], y2n[:])
                t_c1t = pp.tile([3, 512], f32)
                nc.sync.dma_start(t_c1t[:], c1t[:])
                T4s = pp.tile([4, 512], f32)
                nc.vector.tensor_copy(T4s[0:3, :], t_c1t[:])
                nc.sync.dma_start(T4s[3:4, :], ones1[:])
                t_c1r = smp.tile([128, 12], f32, tag="c1r")
                nc.sync.dma_start(t_c1r[:], c1r[:])
                c1sq = smp.tile([128, 12], f32, tag="c1sq")
                nc.scalar.activation(c1sq[:], t_c1r[:], AF.Square)
                r2 = pp.tile([128, 4], f32)
                nc.vector.tensor_reduce(
                    r2[:], c1sq[:].rearrange("p (a b) -> p a b", b=3),
                    axis=AX.X, op=ALU.add)
                rm = pp.tile([128, 16], f32)
                for st in range(4):
                    for pb in range(4):
                        pC = psB.tile([128, 1024], f32, tag="pB")
                        for q in range(2):
                            cs = pb * 2 + q
                            nc.tensor.matmul(
                                pC[:, q * 512:(q + 1) * 512],
                                T4s[:, st * 128:(st + 1) * 128],
                                T4c[0:4, cs * 512:(cs + 1) * 512],
                                start=True, stop=True)
                        nc.vector.tensor_reduce(
                            rm[:, st * 4 + pb:st * 4 + pb + 1],
                            pC[:], axis=AX.X, op=ALU.max)
                gmax = pp.tile([128, 4], f32)
                nc.vector.tensor_reduce(
                    gmax[:], rm[:].rearrange("p (a b) -> p a b", b=4),
                    axis=AX.X, op=ALU.max)
                d2 = pp.tile([128, 4], f32)
                nc.vector.tensor_sub(d2[:], r2[:], gmax[:])
                nc.vector.tensor_scalar_max(d2[:], d2[:], 0.0)
                dd = pp.tile([128, 4], f32)
                nc.scalar.activation(dd[:], d2[:], AF.Sqrt)
                bm = pp.tile([128, 4], f32)
                nc.vector.tensor_scalar(bm[:], dd[:], 0.1, None,
                                        op0=ALU.is_lt)
                dm = pp.tile([128, 4], f32)
                nc.vector.tensor_tensor(dm[:], dd[:], bm[:], op=ALU.mult)
                bsum = pp.tile([128, 1], f32)
                nc.vector.tensor_reduce(bsum[:], dm[:], axis=AX.X,
                                        op=ALU.add)
                bcnt = pp.tile([128, 1], f32)
                nc.vector.tensor_reduce(bcnt[:], bm[:], axis=AX.X,
                                        op=ALU.add)

                # ---------------- output ----------------
                o = pp.tile([128, 8], f32)
                nc.vector.memset(o[:], 0.0)
                nc.vector.tensor_copy(o[:, 0:1], cont_p[:])
                nc.vector.tensor_copy(o[:, 1:2], rec_acc[:])
                nc.vector.tensor_copy(o[:, 2:3], per_acc[:])
                nc.vector.tensor_copy(o[:, 3:4], bsum[:])
                nc.vector.tensor_copy(o[:, 4:5], bcnt[:])
                nc.sync.dma_start(out[:], o[:])
    return nc


def _split_excess_waits(nc, mybir, max_waits=1):
    for fn in nc.m.functions:
        for bb in fn.blocks:
            new_insts = []
            for inst in bb.instructions:
                si = getattr(inst, 'sync_info', None)
                if si is not None and si.on_wait and len(si.on_wait) > max_waits:
                    waits = list(si.on_wait)
                    rest, keep = waits[:-max_waits], waits[-max_waits:]
                    for i in range(0, len(rest), max_waits):
                        nop = mybir.InstNoOp(name=f"{inst.name}-ws{i}")
                        nop.engine = inst.engine
                        nop.sync_info = mybir.SyncInfo(
                            on_wait=rest[i:i + max_waits], on_update=[])
                        new_insts.append(nop)
                    inst.sync_info = mybir.SyncInfo(
                        on_wait=keep,
                        on_update=list(si.on_update) if si.on_update else [])
                new_insts.append(inst)
            bb.instructions = new_insts


class _Compiled:
    def __init__(self, reps=1):
        import jax
        import concourse.mybir as mybir
        from concourse import bass2jax
        from jax.sharding import Mesh, PartitionSpec
        from jax.experimental.shard_map import shard_map

        nc = _build_core(reps)
        _split_excess_waits(nc, mybir)
        bass2jax.install_neuronx_cc_hook()
        partition_name = (nc.partition_id_tensor.name
                          if nc.partition_id_tensor else None)
        in_names, out_names, out_avals = [], [], []
        for alloc in nc.m.functions[0].allocations:
            if not isinstance(alloc, mybir.MemoryLocationSet):
                continue
            name = alloc.memorylocations[0].name
            if alloc.kind == "ExternalInput":
                if name != partition_name:
                    in_names.append(name)
            elif alloc.kind == "ExternalOutput":
                out_names.append(name)
                out_avals.append(jax.core.ShapedArray(
                    tuple(alloc.tensor_shape), mybir.dt.np(alloc.dtype)))
        self.in_names, self.out_names, self.out_avals = \
            in_names, out_names, out_avals
        in_names_all = in_names + out_names
        if partition_name:
            in_names_all.append(partition_name)

        def _body(*args):
            operands = list(args)
            if partition_name is not None:
                operands.append(bass2jax.partition_id_tensor())
            return tuple(bass2jax._bass_exec_p.bind(
                *operands, out_avals=tuple(out_avals),
                in_names=tuple(in_names_all), out_names=tuple(out_names),
                lowering_input_output_aliases=(), sim_require_finite=True,
                sim_require_nnan=True, nc=nc))

        devices = jax.devices()[:8]
        mesh = Mesh(np.asarray(devices), ("core",))
        n_in = len(in_names) + len(out_names)
        self.fn = jax.jit(
            shard_map(_body, mesh=mesh,
                      in_specs=(PartitionSpec("core"),) * n_in,
                      out_specs=(PartitionSpec("core"),) * len(out_names),
                      check_rep=False),
            keep_unused=True)

    def run(self, in_maps):
        concat_in = [np.concatenate([m[n] for m in in_maps], axis=0)
                     for n in self.in_names]
        concat_zeros = [np.zeros((8 * a.shape[0], *a.shape[1:]), a.dtype)
                        for a in self.out_avals]
        outs = self.fn(*concat_in, *concat_zeros)
        outs = [np.asarray(o) for o in outs]
        return [
            {n: outs[i].reshape(8, *self.out_avals[i].shape)[c]
             for i, n in enumerate(self.out_names)}
            for c in range(8)
        ]


def compile_with_reps(reps):
    return _Compiled(reps)


def make_in_maps(predicted, target, predicted_features, target_features,
                 chunk1, chunk2):
    """Pure data-movement sharding of the full inputs into 8 per-core maps."""
    predicted = np.ascontiguousarray(predicted, dtype=np.float32)
    target = np.ascontiguousarray(target, dtype=np.float32)
    in_maps = []
    for core in range(8):
        b, h = core // 2, core % 2
        X = predicted[b]
        perm = np.concatenate([X[h * HALF:(h + 1) * HALF],
                               X[(1 - h) * HALF:(2 - h) * HALF]], axis=0)
        ptb = np.ascontiguousarray(perm.T.reshape(3, 16, 512).reshape(48, 512))
        prow = np.ascontiguousarray(
            predicted[b, h * HALF:(h + 1) * HALF].reshape(128, 96))
        trow_ = np.ascontiguousarray(
            target[b, h * HALF:(h + 1) * HALF].reshape(128, 96))
        if h == 0:
            pf = np.ascontiguousarray(
                predicted_features[b].reshape(128, 8).astype(np.float32))
            tf = np.ascontiguousarray(
                target_features[b].reshape(128, 8).astype(np.float32))
        else:
            pf = np.zeros((128, 8), np.float32)
            tf = np.zeros((128, 8), np.float32)
        c1s = np.asarray(chunk1[core * 512:(core + 1) * 512], np.float32)
        c1t_ = np.ascontiguousarray(c1s.T)
        c1r_ = np.ascontiguousarray(
            c1s.reshape(4, 128, 3).transpose(1, 0, 2).reshape(128, 12))
        c2b_ = np.ascontiguousarray(
            np.asarray(chunk2, np.float32).T.reshape(3, 8, 512)
            .reshape(24, 512))
        in_maps.append({
            "ptb": ptb, "prow": prow, "trow": trow_,
            "pfeat": pf, "tfeat": tf,
            "c1t": c1t_, "c1r": c1r_, "c2b": c2b_,
        })
    return in_maps


def combine(results):
    """Host-side unshard: sum per-core partials -> the 5 output scalars."""
    rec = per = cont = bs = bc = 0.0
    for r in results:
        o = r["out"].astype(np.float64)
        cont += o[:, 0].sum()
        rec += o[:, 1].sum()
        per += o[:, 2].sum()
        bs += o[:, 3].sum()
        bc += o[:, 4].sum()
    recon = rec / (B * N * 3)
    percep = per / (B * 1024)
    cont = cont / (B * N * 8)
    bcr = np.round(bc)
    bnd = bs / max(bcr, 1.0) if bcr > 0 else 0.0
    total = 1.0 * recon + 0.5 * percep + 0.5 * cont + 1.0 * bnd
    return np.array([recon, percep, cont, bnd, total], dtype=np.float32)


def kernel(**inputs):
    global _COMPILED
    if _COMPILED is None:
        _COMPILED = _Compiled()
    in_maps = make_in_maps(**{k: np.asarray(v) for k, v in inputs.items()})
    results = _COMPILED.run(in_maps)
    return combine(results)


if __name__ == "__main__":
    d = np.load("/root/problem/inputs_cache.npz")
    got = kernel(**{k: d[k] for k in d.files})
    exp = np.load("/root/problem/expected_cache.npy")
    print("got:", got)
    print("exp:", exp)
    print("rel:", np.abs(got - exp) / np.maximum(np.abs(exp), 1e-12))
